# revision 15
# baseline (speedup 1.0000x reference)
"""Bass/Trainium2 kernel for the AffineTransformLayer (spatial transformer,
bilinear sampling) problem.

Contract: kernel(X, theta) takes FULL inputs
  X [16, 256, 256, 64] fp32, theta [16, 6] fp32
and returns the FULL output [16, 256, 256, 64] fp32, computing the same
bilinear-sampled affine warp as the reference (including its trunc-cast and
clip edge semantics), data-parallel over 8 NeuronCores (2 samples per core).

Per-core design (two phases):

Phase A — build Z, a bf16 row-pair-interleaved copy of the input in DRAM:
  Z[s, r, x] = [X[s, r, x, 0:64] | X[s, min(r+1,255), x, 0:64]]  (bf16)
  so each 256 B Z entry holds one pixel's channel data for BOTH bilinear
  row taps. Built by streaming 16-row blocks through SBUF with two
  strided ACT-engine (scalar) cast-copies, then one contiguous DMA out.

Phase B — per output pixel a SINGLE dma_gather descriptor (768 B = 3
  consecutive Z entries anchored at an even x-pair) covers the full 2x2
  bilinear footprint for either x-parity; max index 255*128+127 = 32767
  fits int16. This halves the Pool-engine SWDGE descriptor-generation
  work vs a two-row gather, which is the kernel's critical path.
  The weighted sum runs in bf16: per-pixel 6 slot weights (3 x-slots x
  2 rows, zero on unused/degenerate taps) reproduce the reference's
  trunc/clip edge semantics. The weights are expanded to 64-wide on the
  ACT engine so the value multiply uses contiguous operands (2x DVE
  mode); the final sum is cast back to fp32 on ACT.

  Degenerate row case (reference clips y0 and y0+1 to the same row, which
  only happens at the image edges where the paired row in Z differs):
  with d = r1-r0 in {0,1}, use wy1' = wy1*d and wy0' = wy0 + wy1*(1-d),
  exact in both cases.

Index/weight pipelines run in fp32 with arithmetic bit-matching the
reference; ops that are exact under any rounding order (mult/add by a
scalar, pow2 scale-bias, int->float casts) run on the otherwise-idle ACT
engine, which also avoids the SBUF port the DVE shares with GpSimd's
SWDGE descriptor generation. The p-major weight pipeline runs once per
sample [128, 512]; the wrapped index pipeline runs per 32768-pixel
chunk [128, 512]; all pipeline scratch shares one set of tags.
"""

import numpy as np
from contextlib import ExitStack

import concourse.bass as bass
import concourse.tile as tile
from concourse import bacc, mybir
from concourse.bass_utils import run_bass_kernel_spmd

F32 = mybir.dt.float32
BF16 = mybir.dt.bfloat16
I32 = mybir.dt.int32
I16 = mybir.dt.int16
OP = mybir.AluOpType
AF = mybir.ActivationFunctionType

N_CORES = 8
B_PER_CORE = 2
H = W = 256
C = 64
NPIX_S = H * W                 # pixels per sample (65536)
NPIX = B_PER_CORE * NPIX_S     # pixels per core (131072)
SAMPLE_ELEMS = NPIX_S * C      # fp32 elems per sample (4,194,304)

ZROW = W * 2 * C               # bf16 elems per Z row (32768)
ZSAMPLE = H * ZROW             # bf16 elems per Z sample (8,388,608)
ZPAD = 384                     # per-sample tail pad (gather overrun window)
ZSTRIDE = ZSAMPLE + ZPAD

GN = 1024                      # indices per gather instruction
KPG = GN // 128                # free slots per partition per gather (8)
NQ = 4                         # SWDGE queues / gathers per compute group
GROUP = NQ * GN                # pixels per compute group (4096)
CSG = 8                        # compute groups per chunk
CHUNK = CSG * GROUP            # pixels per chunk (32768)
NCHPS = NPIX_S // CHUNK        # chunks per sample (2)
WCOLC = CSG * 64               # wrapped free dim per chunk (512)
FDWS = NPIX_S // 128           # p-major free dim per sample (512)

RB = 8                         # Z rows built per block
NBLK = H // RB                 # blocks per sample (16)

_cached = {}


class _F32View:
    """Present an int32 tile through a bitcast-to-f32 AP via [...]."""

    def __init__(self, t):
        self._t = t

    def __getitem__(self, key):
        return self._t[key].bitcast(F32)


def _trunc(nc, pool, x, tag, act=True):
    """float trunc-toward-zero of fp32 tile x, matching jnp astype(int32):
    trunc(x) = copysign(floor(|x|), x); floor(|x|) = rint(|x|) - (rint > |x|).
    """
    shp = list(x[:].shape)
    ax = pool.tile(shp, I32, tag=f"{tag}_ax")
    nc.vector.tensor_scalar(ax[:], x[:].bitcast(I32), 0x7FFFFFFF, None,
                            OP.bitwise_and)
    axf = ax[:].bitcast(F32)
    ti = pool.tile(shp, I32, tag=f"{tag}_i")
    nc.vector.tensor_copy(ti[:], axf)           # round-to-nearest-even
    tf = pool.tile(shp, F32, tag=f"{tag}_f")
    if act:
        nc.scalar.copy(tf[:], ti[:])            # exact int->float
    else:
        nc.vector.tensor_copy(tf[:], ti[:])
    gt = pool.tile(shp, F32, tag=f"{tag}_gt")
    nc.vector.tensor_tensor(gt[:], tf[:], axf, OP.is_gt)
    fl = pool.tile(shp, F32, tag=f"{tag}_fl")
    nc.vector.tensor_tensor(fl[:], tf[:], gt[:], OP.subtract)
    sgn = pool.tile(shp, I32, tag=f"{tag}_s")
    nc.vector.tensor_scalar(sgn[:], x[:].bitcast(I32), -2147483648, None,
                            OP.bitwise_and)
    out = pool.tile(shp, I32, tag=f"{tag}_o")
    nc.vector.tensor_tensor(out[:], fl[:].bitcast(I32), sgn[:], OP.bitwise_or)
    return _F32View(out)


def _coords(nc, pool, jf, if_, th, s, tag, act=True):
    """px, py from fp32 column/row index tiles, replicating reference
    rounding: xs = j*(2/255) - 1; x_pre = t0*xs + t1*ys + t2;
    px = (x_pre + 1) * 128 (the *0.5*256 of the reference is exact).
    Single-rounding ops (scalar mult/add, pow2 scale+bias) run on ACT."""
    shp = list(jf[:].shape)
    # in-place: jf/if_ are dead after this anyway
    xsv, ysv = jf, if_
    nc.vector.tensor_scalar(xsv[:], jf[:], 2.0 / 255.0, -1.0, OP.mult, OP.add)
    nc.vector.tensor_scalar(ysv[:], if_[:], 2.0 / 255.0, -1.0, OP.mult, OP.add)

    out = []
    for r in range(2):
        c0, c1, c2 = 6 * s + 3 * r, 6 * s + 3 * r + 1, 6 * s + 3 * r + 2
        u1 = pool.tile(shp, F32, tag=f"{tag}_u1")
        if act:
            nc.scalar.mul(u1[:], xsv[:], th[:, c0:c0 + 1])
        else:
            nc.vector.tensor_scalar(u1[:], xsv[:], th[:, c0:c0 + 1], None,
                                    OP.mult)
        u3 = pool.tile(shp, F32, tag=f"{tag}_u3")
        nc.vector.scalar_tensor_tensor(u3[:], ysv[:], th[:, c1:c1 + 1], u1[:],
                                       OP.mult, OP.add)
        u4 = pool.tile(shp, F32, tag=f"{tag}_u4")
        p = pool.tile(shp, F32, tag=f"{tag}_p{r}")
        if act:
            nc.scalar.add(u4[:], u3[:], th[:, c2:c2 + 1])
            nc.scalar.activation(p[:], u4[:], AF.Copy, bias=128.0, scale=128.0)
        else:
            nc.vector.tensor_scalar(u4[:], u3[:], th[:, c2:c2 + 1], None,
                                    OP.add)
            nc.vector.tensor_scalar(p[:], u4[:], 1.0, 128.0, OP.add, OP.mult)
        out.append(p)
    return out


def _clips(nc, pool, v0f, tag, act=True):
    """c0=clip(v0), c1=clip(v0+1) from float trunc tile view v0f."""
    shp = list(v0f[:].shape)
    c0 = pool.tile(shp, F32, tag=f"{tag}_c0")
    nc.vector.tensor_scalar(c0[:], v0f[:], 0.0, 255.0, OP.max, OP.min)
    c1a = pool.tile(shp, F32, tag=f"{tag}_c1a")
    if act:
        nc.scalar.activation(c1a[:], v0f[:], AF.Relu, bias=1.0, scale=1.0)
    else:
        nc.vector.tensor_scalar(c1a[:], v0f[:], 1.0, 0.0, OP.add, OP.max)
    c1 = pool.tile(shp, F32, tag=f"{tag}_c1")
    nc.vector.tensor_scalar(c1[:], c1a[:], 255.0, None, OP.min)
    return c0, c1


def _clip0(nc, pool, v0f, tag):
    """clip(v0) only."""
    shp = list(v0f[:].shape)
    c0 = pool.tile(shp, F32, tag=f"{tag}_c0")
    nc.vector.tensor_scalar(c0[:], v0f[:], 0.0, 255.0, OP.max, OP.min)
    return c0


def _hg(nc, pool, c0, tag, act=True):
    """xg = min(c0, 254), hg = floor(xg/2) as float, both exact."""
    shp = list(c0[:].shape)
    xg = pool.tile(shp, F32, tag=f"{tag}_xg")
    nc.vector.tensor_scalar(xg[:], c0[:], 254.0, None, OP.min)
    xgi = pool.tile(shp, I32, tag=f"{tag}_xgi")
    nc.vector.tensor_copy(xgi[:], xg[:])
    hgi = pool.tile(shp, I32, tag=f"{tag}_hgi")
    nc.vector.tensor_scalar(hgi[:], xgi[:], 1, None, OP.arith_shift_right)
    hgf = pool.tile(shp, F32, tag=f"{tag}_hgf")
    if act:
        nc.scalar.copy(hgf[:], hgi[:])
    else:
        nc.vector.tensor_copy(hgf[:], hgi[:])
    return xg, hgf


def build():
    nc = bacc.Bacc(
        "TRN2",
        target_bir_lowering=False,
        debug=False,
        enable_asserts=False,
        num_devices=N_CORES,
        num_swdge_queues=NQ,
    )
    xp = nc.dram_tensor("xp", [B_PER_CORE * SAMPLE_ELEMS], F32,
                        kind="ExternalInput")
    th_in = nc.dram_tensor("th", [B_PER_CORE, 6], F32, kind="ExternalInput").ap()
    out_d = nc.dram_tensor("out", [NPIX, C], F32, kind="ExternalOutput").ap()
    th_scratch = nc.dram_tensor("th_scratch", [B_PER_CORE, 6], F32).ap()
    z = nc.dram_tensor("z", [B_PER_CORE * ZSTRIDE], BF16)

    zaps = [
        bass.AP(z, s * ZSTRIDE, [[256, 32768], [1, 384]])
        for s in range(B_PER_CORE)
    ]

    with tile.TileContext(nc) as tc, ExitStack() as ctx:
        pers = ctx.enter_context(tc.tile_pool(name="pers", bufs=1))

        # ---- theta -> [128, 12] broadcast tile ----
        th_sb = pers.tile([B_PER_CORE, 6], F32)
        nc.sync.dma_start(th_sb[:], th_in[:])
        nc.sync.dma_start(th_scratch[:], th_sb[:])
        th = pers.tile([128, 12], F32)
        th_bc_src = bass.AP(th_scratch.tensor, 0, [[0, 128], [1, 12]])
        nc.sync.dma_start(th[:], th_bc_src)

        # ---- zero the per-sample Z tail pads (gather overrun windows;
        # must be finite before ANY gather since 0-weight slots still
        # multiply the gathered bytes) ----
        zp = pers.tile([128, 3], BF16)
        nc.vector.memset(zp[:], 0)
        for s in range(B_PER_CORE):
            zpad_ap = bass.AP(z, s * ZSTRIDE + ZSAMPLE, [[3, 128], [1, 3]])
            nc.sync.dma_start(zpad_ap, zp[:])

        # ---- per-partition constants ----
        pidx = pers.tile([128, 1], I32)
        nc.gpsimd.iota(pidx[:], pattern=[[0, 1]], base=0, channel_multiplier=1)
        p16 = pers.tile([128, 1], I32)
        nc.vector.tensor_scalar(p16[:], pidx[:], 15, None, OP.bitwise_and)
        p32 = pers.tile([128, 1], I32)
        nc.vector.tensor_scalar(p32[:], pidx[:], 5, None, OP.arith_shift_right)
        pj16i = pers.tile([128, 1], I32)
        nc.vector.tensor_scalar(pj16i[:], p16[:], 3, None, OP.logical_shift_left)
        pj16f = pers.tile([128, 1], F32)
        nc.vector.tensor_copy(pj16f[:], pj16i[:])
        pg4i = pers.tile([128, 1], I32)
        nc.vector.tensor_scalar(pg4i[:], p32[:], 2, None, OP.logical_shift_left)
        pg4f = pers.tile([128, 1], F32)
        nc.vector.tensor_copy(pg4f[:], pg4i[:])
        p32m = pers.tile([128, 1], I32)
        nc.vector.tensor_scalar(p32m[:], pidx[:], 31, None, OP.bitwise_and)
        pp32i = pers.tile([128, 1], I32)
        nc.vector.tensor_scalar(pp32i[:], p32m[:], 3, None, OP.logical_shift_left)
        pp32f = pers.tile([128, 1], F32)
        nc.vector.tensor_copy(pp32f[:], pp32i[:])
        pg1f = pers.tile([128, 1], F32)
        nc.vector.tensor_copy(pg1f[:], p32[:])
        # wrapped i-base per chunk: pg4f + 128*chunk_half (precomputed)
        wib = []
        for ch in range(NCHPS):
            t = pers.tile([128, 1], F32, tag=f"wib{ch}")
            nc.vector.tensor_scalar(t[:], pg4f[:], float(128 * ch), None, OP.add)
            wib.append(t)

        # ---- hoisted iota bases (sample-independent) ----
        # wrapped: col = csg*64 + ci*8 + cqh*2 + cql
        # j = 128*cql + 8*(p%16) + ci ; i = 128*ch + 16*csg + 4*(p//32) + cqh
        wj0 = pers.tile([128, WCOLC], F32)
        nc.gpsimd.iota(wj0[:], pattern=[[0, CSG], [1, 8], [0, 4], [128, 2]],
                       base=0, channel_multiplier=0,
                       allow_small_or_imprecise_dtypes=True)
        wi0 = pers.tile([128, WCOLC], F32)
        nc.gpsimd.iota(wi0[:], pattern=[[16, CSG], [0, 8], [1, 4], [0, 2]],
                       base=0, channel_multiplier=0,
                       allow_small_or_imprecise_dtypes=True)
        # p-major (whole sample): col = ch*8 + k
        # j = 8*(p%32) + k ; i = 4*ch + p//32
        pj0 = pers.tile([128, FDWS], F32)
        nc.gpsimd.iota(pj0[:], pattern=[[0, FDWS // KPG], [1, KPG]],
                       base=0, channel_multiplier=0,
                       allow_small_or_imprecise_dtypes=True)
        pi0 = pers.tile([128, FDWS], F32)
        nc.gpsimd.iota(pi0[:], pattern=[[4, FDWS // KPG], [0, KPG]],
                       base=0, channel_multiplier=0,
                       allow_small_or_imprecise_dtypes=True)


        # ---- pools: phase-B pools allocated FIRST so the Z-build scratch
        # (zb) is address-disjoint — SBUF-reuse anti-deps would otherwise
        # serialize all of phase B behind the Z build ----
        wp = ctx.enter_context(tc.tile_pool(name="wp", bufs=1))
        ip = ctx.enter_context(tc.tile_pool(name="ip", bufs=2))     # w6d
        dp = ctx.enter_context(tc.tile_pool(name="dp", bufs=3))     # idxa
        gp = ctx.enter_context(tc.tile_pool(name="gp", bufs=2))     # ta groups
        fp = ctx.enter_context(tc.tile_pool(name="fp", bufs=2))     # f1
        op_ = ctx.enter_context(tc.tile_pool(name="op", bufs=2))    # ot
        zb = ctx.enter_context(tc.tile_pool(name="zb", bufs=2))



        # ================= Phase A: build Z (Sync DMA + ACT casts) =========
        def z_block(s, blk):
                rb = blk * RB
                last = blk == NBLK - 1
                nrows = RB if last else RB + 1
                tin = zb.tile([128, (RB + 1) * 128], F32, tag="tin")
                src = bass.AP(xp, s * SAMPLE_ELEMS + rb * W * C,
                              [[128, 128], [W * C, nrows], [1, 128]])
                tin_v = tin[:, 0:nrows * 128].rearrange(
                    "p (r q) -> p r q", r=nrows)
                nc.sync.dma_start(tin_v, src)
                if last:
                    # duplicate row 255 into slot RB (clip row)
                    dup = bass.AP(xp, s * SAMPLE_ELEMS + 255 * W * C,
                                  [[128, 128], [1, 128]])
                    nc.sync.dma_start(tin[:, RB * 128:(RB + 1) * 128], dup)
                zt = zb.tile([128, RB * 256], BF16, tag="zt")
                zt_t0 = bass.AP(zt[:].tensor, zt[:].offset,
                                [zt[:].ap[0], [256, RB], [128, 2], [1, 64]])
                zt_t1 = bass.AP(zt[:].tensor, zt[:].offset + 64,
                                [zt[:].ap[0], [256, RB], [128, 2], [1, 64]])
                src_r0 = bass.AP(tin[:].tensor, tin[:].offset,
                                 [tin[:].ap[0], [128, RB], [64, 2], [1, 64]])
                src_r1 = bass.AP(tin[:].tensor, tin[:].offset + 128,
                                 [tin[:].ap[0], [128, RB], [64, 2], [1, 64]])
                nc.scalar.copy(zt_t0, src_r0)
                nc.scalar.copy(zt_t1, src_r1)
                zdst = bass.AP(z, s * ZSTRIDE + rb * ZROW,
                               [[256, 128], [ZROW, RB], [1, 256]])
                nc.scalar.dma_start(zdst, zt[:].rearrange(
                    "p (r q) -> p r q", r=RB))

        # ========== index/weight pipelines, all samples (DVE, upfront) =====
        # These run on the otherwise-idle DVE while the Z build streams
        # through Sync/ACT, so the Pool engine can start gathering the
        # moment sample 0's Z lands.

        def wrapped_pipe(s, ch):
            """Index pipeline for one chunk -> idxa tile. All-DVE (it gates
            the Pool gathers; engine crossings would add latency).

            Degenerate pixels (x-taps or y-taps clip to the same coord) have
            all-zero weights, so their gathered bytes are irrelevant — but
            clipped coords concentrate up to ~60% of a sample's descriptors
            onto ONE 768B entry, serializing a single HBM channel (measured
            2-3x slowdown on heavily-clipped samples). Redirect them to the
            identity-map entry (i_img*128 + own-xpair): uniform, sequential-
            friendly addresses."""
            wjf = wp.tile([128, WCOLC], F32, tag="jf")
            nc.vector.tensor_scalar(wjf[:], wj0[:], pj16f[:, 0:1], None,
                                    OP.add)
            wif = wp.tile([128, WCOLC], F32, tag="if")
            nc.vector.tensor_scalar(wif[:], wi0[:], wib[ch][:, 0:1], None,
                                    OP.add)
            wji = wp.tile([128, WCOLC], I32, tag="t_i")
            nc.vector.tensor_copy(wji[:], wjf[:])
            wjh = wp.tile([128, WCOLC], I32, tag="t_ax")
            nc.vector.tensor_scalar(wjh[:], wji[:], 1, None,
                                    OP.arith_shift_right)
            jhf = wp.tile([128, WCOLC], F32, tag="t_f")
            nc.vector.tensor_copy(jhf[:], wjh[:])
            jnk = wp.tile([128, WCOLC], F32, tag="wx1")
            nc.vector.scalar_tensor_tensor(jnk[:], wif[:], 128.0, jhf[:],
                                           OP.mult, OP.add)

            wpx, wpy = _coords(nc, wp, wjf, wif, th, s, "c", act=False)
            wx0f = _trunc(nc, wp, wpx, "t", act=False)
            wc0, wc1 = _clips(nc, wp, wx0f, "cx", act=False)
            _wxg, whgf = _hg(nc, wp, wc0, "hg", act=False)
            wy0f = _trunc(nc, wp, wpy, "t", act=False)
            wr0, wr1 = _clips(nc, wp, wy0f, "cy", act=False)

            gx = wp.tile([128, WCOLC], F32, tag="q")
            nc.vector.tensor_tensor(gx[:], wc1[:], wc0[:], OP.subtract)
            dy = wp.tile([128, WCOLC], F32, tag="g_")
            nc.vector.tensor_tensor(dy[:], wr1[:], wr0[:], OP.subtract)
            m = wp.tile([128, WCOLC], F32, tag="m")
            nc.vector.tensor_tensor(m[:], gx[:], dy[:], OP.mult)

            real = wp.tile([128, WCOLC], F32, tag="wx0a")
            nc.vector.scalar_tensor_tensor(real[:], wr0[:], 128.0, whgf[:],
                                           OP.mult, OP.add)
            dfj = wp.tile([128, WCOLC], F32, tag="wx1a")
            nc.vector.tensor_tensor(dfj[:], real[:], jnk[:], OP.subtract)
            mdf = wp.tile([128, WCOLC], F32, tag="wx0")
            nc.vector.tensor_tensor(mdf[:], m[:], dfj[:], OP.mult)
            idxa = dp.tile([128, WCOLC], I16, tag="idxa")
            nc.vector.tensor_tensor(idxa[:], mdf[:], jnk[:], OP.add)
            return idxa

        def pmajor_pipe(s):
            """Weight pipeline for a whole sample [128, FDWS] -> w6d."""
            pjf = wp.tile([128, FDWS], F32, tag="jf")
            nc.vector.tensor_scalar(pjf[:], pj0[:], pp32f[:, 0:1], None, OP.add)
            pif = wp.tile([128, FDWS], F32, tag="if")
            nc.vector.tensor_scalar(pif[:], pi0[:], pg1f[:, 0:1], None, OP.add)

            ppx, ppy = _coords(nc, wp, pjf, pif, th, s, "c", act=False)
            px0f = _trunc(nc, wp, ppx, "t", act=False)
            c0, c1 = _clips(nc, wp, px0f, "cx", act=False)
            xg, hgf = _hg(nc, wp, c0, "hg", act=False)
            py0f = _trunc(nc, wp, ppy, "t", act=False)
            r0, r1 = _clips(nc, wp, py0f, "cy", act=False)

            q = wp.tile([128, FDWS], F32, tag="q")
            nc.vector.scalar_tensor_tensor(q[:], hgf[:], -2.0, xg[:],
                                           OP.mult, OP.add)

            g_ = wp.tile([128, FDWS], F32, tag="g_")
            nc.vector.tensor_tensor(g_[:], c1[:], c0[:], OP.subtract)
            wx0a = wp.tile([128, FDWS], F32, tag="wx0a")
            nc.vector.tensor_tensor(wx0a[:], c1[:], ppx[:], OP.subtract)
            wx0 = wp.tile([128, FDWS], F32, tag="wx0")
            nc.vector.tensor_tensor(wx0[:], wx0a[:], g_[:], OP.mult)
            wx1a = wp.tile([128, FDWS], F32, tag="wx1a")
            nc.vector.tensor_tensor(wx1a[:], ppx[:], c0[:], OP.subtract)
            wx1 = wp.tile([128, FDWS], F32, tag="wx1")
            nc.vector.tensor_tensor(wx1[:], wx1a[:], g_[:], OP.mult)

            # q is exactly 0.0 or 1.0, so wx*(1-q) == wx - wx*q bit-for-bit.
            w012 = wp.tile([128, FDWS, 3], F32, tag="w012")
            nc.vector.tensor_tensor(w012[:, :, 1:2], wx0[:].unsqueeze(-1),
                                    q[:].unsqueeze(-1), OP.mult)
            nc.vector.tensor_tensor(w012[:, :, 2:3], wx1[:].unsqueeze(-1),
                                    q[:].unsqueeze(-1), OP.mult)
            nc.vector.tensor_tensor(w012[:, :, 0:1], wx0[:].unsqueeze(-1),
                                    w012[:, :, 1:2], OP.subtract)
            w1b = wp.tile([128, FDWS], F32, tag="w1b")
            nc.vector.tensor_tensor(w1b[:], wx1[:], w012[:, :, 2], OP.subtract)
            nc.vector.tensor_tensor(w012[:, :, 1:2], w012[:, :, 1:2],
                                    w1b[:].unsqueeze(-1), OP.add)

            # y weights with degenerate-row (r1==r0) redistribution:
            # d = r1-r0 in {0,1}; wy1' = wy1*d ; wy0' = wy0 + wy1*(1-d)
            wy0 = wp.tile([128, FDWS], F32, tag="wy0")
            nc.vector.tensor_tensor(wy0[:], r1[:], ppy[:], OP.subtract)
            wy1 = wp.tile([128, FDWS], F32, tag="wy1")
            nc.vector.tensor_tensor(wy1[:], ppy[:], r0[:], OP.subtract)
            d = wp.tile([128, FDWS], F32, tag="d")
            nc.vector.tensor_tensor(d[:], r1[:], r0[:], OP.subtract)
            e = wp.tile([128, FDWS], F32, tag="e")
            nc.vector.tensor_scalar(e[:], d[:], -1.0, 1.0, OP.mult, OP.add)
            m = wp.tile([128, FDWS], F32, tag="m")
            nc.vector.tensor_tensor(m[:], wy1[:], e[:], OP.mult)
            wy0p = wp.tile([128, FDWS], F32, tag="wy0p")
            nc.vector.tensor_tensor(wy0p[:], wy0[:], m[:], OP.add)
            wy1p = wp.tile([128, FDWS], F32, tag="wy1p")
            nc.vector.tensor_tensor(wy1p[:], wy1[:], d[:], OP.mult)

            # w6d[p, col, s*2+t, dup2] = w012[s] * wy't  (bf16, duplicated
            # pairwise so the value multiply's innermost dim is a unit-stride
            # 2-elem run on BOTH operands -> DVE 2x_1P mode)
            w6d = ip.tile([128, FDWS, 6, 2], BF16, tag="w6d")
            w6_t0 = bass.AP(w6d[:].tensor, w6d[:].offset,
                            [w6d[:].ap[0], [12, FDWS], [4, 3], [1, 2]])
            w6_t1 = bass.AP(w6d[:].tensor, w6d[:].offset + 2,
                            [w6d[:].ap[0], [12, FDWS], [4, 3], [1, 2]])
            w012_bc = bass.AP(w012[:].tensor, w012[:].offset,
                              [w012[:].ap[0], [3, FDWS], [1, 3], [0, 2]])
            wy0_bc = bass.AP(wy0p[:].tensor, wy0p[:].offset,
                             [wy0p[:].ap[0], [1, FDWS], [0, 3], [0, 2]])
            wy1_bc = bass.AP(wy1p[:].tensor, wy1p[:].offset,
                             [wy1p[:].ap[0], [1, FDWS], [0, 3], [0, 2]])
            nc.vector.tensor_tensor(w6_t0, w012_bc, wy0_bc, OP.mult)
            nc.vector.tensor_tensor(w6_t1, w012_bc, wy1_bc, OP.mult)
            return w6d

        # Issue order: Z(s0) blocks; s0 pipelines (DVE overlaps the Z-s0
        # DMA/casts); s0-ch0 compute with Z(s1) blocks interleaved into the
        # stream (so Z-s1's Sync/ACT queue entries don't head-of-line block
        # s0's output DMAs); s1 pipelines; remaining compute.
        for blk in range(NBLK):
            z_block(0, blk)

        idxas = {}
        idxas[(0, 0)] = wrapped_pipe(0, 0)
        w6d0 = pmajor_pipe(0)
        idxas[(0, 1)] = wrapped_pipe(0, 1)
        w6ds = {0: w6d0}

        zq = [(1, blk) for blk in range(NBLK)]  # remaining Z blocks

        def cs_group(s, ch, cs):
            cbase = s * NPIX_S + ch * CHUNK
            idxa = idxas[(s, ch)]
            w6d = w6ds[s]
            cs_s = ch * CSG + cs
            ta = gp.tile([128, 4 * KPG, 384], BF16, tag="ta")
            for g in range(4):
                nc.gpsimd.dma_gather(
                    out_ap=ta[:, g * KPG:(g + 1) * KPG, :],
                    in_ap=zaps[s],
                    idxs_ap=idxa[:, cs * 64:(cs + 1) * 64],
                    num_idxs=GN, num_idxs_reg=GN,
                    elem_size=384, elem_step=256, queue_num=g,
                )
            # weight multiply at 2x: innermost 2-elem unit-stride run
            # on both operands (w6d is pair-duplicated)
            va5 = ta[:].rearrange("p k (sl c2 pr) -> p k sl c2 pr",
                                  sl=6, pr=2)
            lo = cs_s * 32
            wsl = w6d[:, lo:lo + 32, :, :]
            w_bc = bass.AP(wsl.tensor, wsl.offset,
                           [wsl.ap[0], [12, 32], [2, 6], [0, 32], [1, 2]])
            nc.vector.tensor_tensor(va5, va5, w_bc, OP.mult)
            # reduce: t-pairs in place, then s-slots into f1 (frees ta
            # at the last DVE op — no ACT crossing holds the buffer)
            pdim = ta[:].ap[0]
            toff = ta[:].offset
            t0v = bass.AP(ta[:].tensor, toff,
                          [pdim, [384, 4 * KPG], [128, 3], [1, 64]])
            t1v = bass.AP(ta[:].tensor, toff + 64,
                          [pdim, [384, 4 * KPG], [128, 3], [1, 64]])
            nc.vector.tensor_tensor(t0v, t0v, t1v, OP.add)
            a0 = bass.AP(ta[:].tensor, toff,
                         [pdim, [384, 4 * KPG], [1, 64]])
            a1 = bass.AP(ta[:].tensor, toff + 128,
                         [pdim, [384, 4 * KPG], [1, 64]])
            a2 = bass.AP(ta[:].tensor, toff + 256,
                         [pdim, [384, 4 * KPG], [1, 64]])
            nc.vector.tensor_tensor(a0, a0, a1, OP.add)
            f1 = fp.tile([128, 4 * KPG, 64], BF16, tag="f1")
            nc.vector.tensor_tensor(f1[:], a0, a2, OP.add)
            ot = op_.tile([128, 4 * KPG, 64], F32, tag="ot")
            nc.scalar.copy(ot[:], f1[:])

            # out rows: cbase + cs*GROUP + g*GN + p*KPG + k
            obase = cbase + cs * GROUP
            oap = bass.AP(out_d.tensor, obase * C,
                          [[KPG * C, 128], [GN * C, 4], [1, KPG * C]])
            ovw = ot[:].rearrange("p (g k) c -> p g (k c)", g=4)
            nc.scalar.dma_start(oap, ovw)

        zper = (len(zq) + CSG - 1) // CSG
        for cs in range(CSG):
            cs_group(0, 0, cs)
            for _ in range(zper):
                if zq:
                    z_block(*zq.pop(0))

        idxas[(1, 0)] = wrapped_pipe(1, 0)
        w6ds[1] = pmajor_pipe(1)
        idxas[(1, 1)] = wrapped_pipe(1, 1)

        for cs in range(CSG):
            cs_group(0, 1, cs)
        for ch in range(NCHPS):
            for cs in range(CSG):
                cs_group(1, ch, cs)

    nc.compile()
    return nc


def _get_runner():
    """Build once: the Bass program, the sharded jitted executor, and the
    device-resident zero output buffers. Cached for repeat kernel() calls."""
    if "runner" in _cached:
        return _cached["runner"]

    import jax
    from jax.sharding import Mesh, PartitionSpec, NamedSharding
    from jax.experimental.shard_map import shard_map
    import concourse.bass2jax as bass2jax

    nc = build()
    bass2jax.install_neuronx_cc_hook()

    in_names, out_names, out_avals, zero_outs = [], [], [], []
    pn = nc.partition_id_tensor.name if nc.partition_id_tensor else None
    for alloc in nc.m.functions[0].allocations:
        if not isinstance(alloc, mybir.MemoryLocationSet):
            continue
        name = alloc.memorylocations[0].name
        if alloc.kind == "ExternalInput":
            if name != pn:
                in_names.append(name)
        elif alloc.kind == "ExternalOutput":
            out_names.append(name)
            shape = tuple(alloc.tensor_shape)
            dtype = mybir.dt.np(alloc.dtype)
            out_avals.append(jax.core.ShapedArray(shape, dtype))
            zero_outs.append(np.zeros(shape, dtype))

    def _body(*args):
        ops = list(args)
        if pn is not None:
            ops.append(bass2jax.partition_id_tensor())
        return tuple(bass2jax._bass_exec_p.bind(
            *ops,
            out_avals=tuple(out_avals),
            in_names=tuple(list(in_names) + out_names + ([pn] if pn else [])),
            out_names=tuple(out_names),
            lowering_input_output_aliases=(),
            sim_require_finite=True,
            sim_require_nnan=True,
            nc=nc,
        ))

    devices = jax.devices()[:N_CORES]
    mesh = Mesh(np.asarray(devices), ("core",))
    nin = len(in_names) + len(out_names)
    fn = jax.jit(
        shard_map(_body, mesh=mesh, in_specs=(PartitionSpec("core"),) * nin,
                  out_specs=(PartitionSpec("core"),) * len(out_names),
                  check_rep=False),
        keep_unused=True,
    )
    sh = NamedSharding(mesh, PartitionSpec("core"))
    dz = [jax.device_put(np.zeros((N_CORES * z.shape[0], *z.shape[1:]), z.dtype), sh)
          for z in zero_outs]
    runner = {
        "fn": fn, "dz": dz, "sh": sh, "in_names": in_names,
        "out_idx": out_names.index("out"), "device_put": jax.device_put,
        "nc": nc,
    }
    _cached["runner"] = runner
    return runner


def kernel(X, theta):
    X = np.ascontiguousarray(X, dtype=np.float32)
    theta = np.ascontiguousarray(theta, dtype=np.float32)
    r = _get_runner()
    per_name = {
        "xp": X.reshape(N_CORES, B_PER_CORE * SAMPLE_ELEMS).reshape(-1),
        "th": theta.reshape(N_CORES * B_PER_CORE, 6),
    }
    di = [r["device_put"](per_name[nm], r["sh"]) for nm in r["in_names"]]
    out = r["fn"](*di, *r["dz"])
    res = np.asarray(out[r["out_idx"]])
    return res.reshape(N_CORES * B_PER_CORE, H, W, C)



# revision 18
# speedup vs baseline: 1.3622x; 1.3622x over previous
"""Bass/Trainium2 kernel for the AffineTransformLayer (spatial transformer,
bilinear sampling) problem.

Contract: kernel(X, theta) takes FULL inputs
  X [16, 256, 256, 64] fp32, theta [16, 6] fp32
and returns the FULL output [16, 256, 256, 64] fp32, computing the same
bilinear-sampled affine warp as the reference (including its trunc-cast and
clip edge semantics), data-parallel over 8 NeuronCores (2 samples per core).

Per-core design (two phases):

Phase A — build Z, a bf16 row-pair-interleaved copy of the input in DRAM:
  Z[s, r, x] = [X[s, r, x, 0:64] | X[s, min(r+1,255), x, 0:64]]  (bf16)
  so each 256 B Z entry holds one pixel's channel data for BOTH bilinear
  row taps. Built by streaming 16-row blocks through SBUF with two
  strided ACT-engine (scalar) cast-copies, then one contiguous DMA out.

Phase B — per output pixel a SINGLE dma_gather descriptor (768 B = 3
  consecutive Z entries anchored at an even x-pair) covers the full 2x2
  bilinear footprint for either x-parity; max index 255*128+127 = 32767
  fits int16. This halves the Pool-engine SWDGE descriptor-generation
  work vs a two-row gather, which is the kernel's critical path.
  The weighted sum runs in bf16: per-pixel 6 slot weights (3 x-slots x
  2 rows, zero on unused/degenerate taps) reproduce the reference's
  trunc/clip edge semantics. The weights are expanded to 64-wide on the
  ACT engine so the value multiply uses contiguous operands (2x DVE
  mode); the final sum is cast back to fp32 on ACT.

  Degenerate row case (reference clips y0 and y0+1 to the same row, which
  only happens at the image edges where the paired row in Z differs):
  with d = r1-r0 in {0,1}, use wy1' = wy1*d and wy0' = wy0 + wy1*(1-d),
  exact in both cases.

Index/weight pipelines run in fp32 with arithmetic bit-matching the
reference; ops that are exact under any rounding order (mult/add by a
scalar, pow2 scale-bias, int->float casts) run on the otherwise-idle ACT
engine, which also avoids the SBUF port the DVE shares with GpSimd's
SWDGE descriptor generation. The p-major weight pipeline runs once per
sample [128, 512]; the wrapped index pipeline runs per 32768-pixel
chunk [128, 512]; all pipeline scratch shares one set of tags.
"""

import numpy as np
from contextlib import ExitStack

import concourse.bass as bass
import concourse.tile as tile
from concourse import bacc, mybir
from concourse.bass_utils import run_bass_kernel_spmd

F32 = mybir.dt.float32
BF16 = mybir.dt.bfloat16
I32 = mybir.dt.int32
I16 = mybir.dt.int16
OP = mybir.AluOpType
AF = mybir.ActivationFunctionType

N_CORES = 8
B_PER_CORE = 2
H = W = 256
C = 64
NPIX_S = H * W                 # pixels per sample (65536)
NPIX = B_PER_CORE * NPIX_S     # pixels per core (131072)
SAMPLE_ELEMS = NPIX_S * C      # fp32 elems per sample (4,194,304)

ZROW = W * 2 * C               # bf16 elems per Z row (32768)
ZSAMPLE = H * ZROW             # bf16 elems per Z sample (8,388,608)
ZPAD = 384                     # per-sample tail pad (gather overrun window)
ZSTRIDE = ZSAMPLE + ZPAD

GN = 1024                      # indices per gather instruction
KPG = GN // 128                # free slots per partition per gather (8)
NQ = 4                         # SWDGE queues / gathers per compute group
GROUP = NQ * GN                # pixels per compute group (4096)
CSG = 8                        # compute groups per chunk
CHUNK = CSG * GROUP            # pixels per chunk (32768)
NCHPS = NPIX_S // CHUNK        # chunks per sample (2)
WCOLC = CSG * 64               # wrapped free dim per chunk (512)
FDWS = NPIX_S // 128           # p-major free dim per sample (512)

RB = 8                         # Z rows built per block
NBLK = H // RB                 # blocks per sample (16)

_cached = {}


class _F32View:
    """Present an int32 tile through a bitcast-to-f32 AP via [...]."""

    def __init__(self, t):
        self._t = t

    def __getitem__(self, key):
        return self._t[key].bitcast(F32)


def _trunc(nc, pool, x, tag, act=True):
    """float trunc-toward-zero of fp32 tile x, matching jnp astype(int32):
    trunc(x) = copysign(floor(|x|), x); floor(|x|) = rint(|x|) - (rint > |x|).
    """
    shp = list(x[:].shape)
    ax = pool.tile(shp, I32, tag=f"{tag}_ax")
    nc.vector.tensor_scalar(ax[:], x[:].bitcast(I32), 0x7FFFFFFF, None,
                            OP.bitwise_and)
    axf = ax[:].bitcast(F32)
    ti = pool.tile(shp, I32, tag=f"{tag}_i")
    nc.vector.tensor_copy(ti[:], axf)           # round-to-nearest-even
    tf = pool.tile(shp, F32, tag=f"{tag}_f")
    if act:
        nc.scalar.copy(tf[:], ti[:])            # exact int->float
    else:
        nc.vector.tensor_copy(tf[:], ti[:])
    gt = pool.tile(shp, F32, tag=f"{tag}_gt")
    nc.vector.tensor_tensor(gt[:], tf[:], axf, OP.is_gt)
    fl = pool.tile(shp, F32, tag=f"{tag}_fl")
    nc.vector.tensor_tensor(fl[:], tf[:], gt[:], OP.subtract)
    sgn = pool.tile(shp, I32, tag=f"{tag}_s")
    nc.vector.tensor_scalar(sgn[:], x[:].bitcast(I32), -2147483648, None,
                            OP.bitwise_and)
    out = pool.tile(shp, I32, tag=f"{tag}_o")
    nc.vector.tensor_tensor(out[:], fl[:].bitcast(I32), sgn[:], OP.bitwise_or)
    return _F32View(out)


def _coords(nc, pool, jf, if_, th, s, tag, act=True):
    """px, py from fp32 column/row index tiles, replicating reference
    rounding: xs = j*(2/255) - 1; x_pre = t0*xs + t1*ys + t2;
    px = (x_pre + 1) * 128 (the *0.5*256 of the reference is exact).
    Single-rounding ops (scalar mult/add, pow2 scale+bias) run on ACT."""
    shp = list(jf[:].shape)
    # in-place: jf/if_ are dead after this anyway
    xsv, ysv = jf, if_
    nc.vector.tensor_scalar(xsv[:], jf[:], 2.0 / 255.0, -1.0, OP.mult, OP.add)
    nc.vector.tensor_scalar(ysv[:], if_[:], 2.0 / 255.0, -1.0, OP.mult, OP.add)

    out = []
    for r in range(2):
        c0, c1, c2 = 6 * s + 3 * r, 6 * s + 3 * r + 1, 6 * s + 3 * r + 2
        u1 = pool.tile(shp, F32, tag=f"{tag}_u1")
        if act:
            nc.scalar.mul(u1[:], xsv[:], th[:, c0:c0 + 1])
        else:
            nc.vector.tensor_scalar(u1[:], xsv[:], th[:, c0:c0 + 1], None,
                                    OP.mult)
        u3 = pool.tile(shp, F32, tag=f"{tag}_u3")
        nc.vector.scalar_tensor_tensor(u3[:], ysv[:], th[:, c1:c1 + 1], u1[:],
                                       OP.mult, OP.add)
        u4 = pool.tile(shp, F32, tag=f"{tag}_u4")
        p = pool.tile(shp, F32, tag=f"{tag}_p{r}")
        if act:
            nc.scalar.add(u4[:], u3[:], th[:, c2:c2 + 1])
            nc.scalar.activation(p[:], u4[:], AF.Copy, bias=128.0, scale=128.0)
        else:
            nc.vector.tensor_scalar(u4[:], u3[:], th[:, c2:c2 + 1], None,
                                    OP.add)
            nc.vector.tensor_scalar(p[:], u4[:], 1.0, 128.0, OP.add, OP.mult)
        out.append(p)
    return out


def _clips(nc, pool, v0f, tag, act=True):
    """c0=clip(v0), c1=clip(v0+1) from float trunc tile view v0f."""
    shp = list(v0f[:].shape)
    c0 = pool.tile(shp, F32, tag=f"{tag}_c0")
    nc.vector.tensor_scalar(c0[:], v0f[:], 0.0, 255.0, OP.max, OP.min)
    c1a = pool.tile(shp, F32, tag=f"{tag}_c1a")
    if act:
        nc.scalar.activation(c1a[:], v0f[:], AF.Relu, bias=1.0, scale=1.0)
    else:
        nc.vector.tensor_scalar(c1a[:], v0f[:], 1.0, 0.0, OP.add, OP.max)
    c1 = pool.tile(shp, F32, tag=f"{tag}_c1")
    nc.vector.tensor_scalar(c1[:], c1a[:], 255.0, None, OP.min)
    return c0, c1


def _clip0(nc, pool, v0f, tag):
    """clip(v0) only."""
    shp = list(v0f[:].shape)
    c0 = pool.tile(shp, F32, tag=f"{tag}_c0")
    nc.vector.tensor_scalar(c0[:], v0f[:], 0.0, 255.0, OP.max, OP.min)
    return c0


def _hg(nc, pool, c0, tag, act=True):
    """xg = min(c0, 254), hg = floor(xg/2) as float, both exact."""
    shp = list(c0[:].shape)
    xg = pool.tile(shp, F32, tag=f"{tag}_xg")
    nc.vector.tensor_scalar(xg[:], c0[:], 254.0, None, OP.min)
    xgi = pool.tile(shp, I32, tag=f"{tag}_xgi")
    nc.vector.tensor_copy(xgi[:], xg[:])
    hgi = pool.tile(shp, I32, tag=f"{tag}_hgi")
    nc.vector.tensor_scalar(hgi[:], xgi[:], 1, None, OP.arith_shift_right)
    hgf = pool.tile(shp, F32, tag=f"{tag}_hgf")
    if act:
        nc.scalar.copy(hgf[:], hgi[:])
    else:
        nc.vector.tensor_copy(hgf[:], hgi[:])
    return xg, hgf


def build():
    nc = bacc.Bacc(
        "TRN2",
        target_bir_lowering=False,
        debug=False,
        enable_asserts=False,
        num_devices=N_CORES,
        num_swdge_queues=NQ,
    )
    xp = nc.dram_tensor("xp", [B_PER_CORE * SAMPLE_ELEMS], F32,
                        kind="ExternalInput")
    th_in = nc.dram_tensor("th", [B_PER_CORE, 6], F32, kind="ExternalInput").ap()
    out_d = nc.dram_tensor("out", [NPIX, C], F32, kind="ExternalOutput").ap()
    th_scratch = nc.dram_tensor("th_scratch", [B_PER_CORE, 6], F32).ap()
    z = nc.dram_tensor("z", [B_PER_CORE * ZSTRIDE], BF16)

    zaps = [
        bass.AP(z, s * ZSTRIDE, [[256, 32768], [1, 384]])
        for s in range(B_PER_CORE)
    ]

    with tile.TileContext(nc) as tc, ExitStack() as ctx:
        pers = ctx.enter_context(tc.tile_pool(name="pers", bufs=1))

        # ---- theta -> [128, 12] broadcast tile ----
        th_sb = pers.tile([B_PER_CORE, 6], F32)
        nc.sync.dma_start(th_sb[:], th_in[:])
        nc.sync.dma_start(th_scratch[:], th_sb[:])
        th = pers.tile([128, 12], F32)
        th_bc_src = bass.AP(th_scratch.tensor, 0, [[0, 128], [1, 12]])
        nc.sync.dma_start(th[:], th_bc_src)

        # ---- zero the per-sample Z tail pads (gather overrun windows;
        # must be finite before ANY gather since 0-weight slots still
        # multiply the gathered bytes) ----
        zp = pers.tile([128, 3], BF16)
        nc.vector.memset(zp[:], 0)
        for s in range(B_PER_CORE):
            zpad_ap = bass.AP(z, s * ZSTRIDE + ZSAMPLE, [[3, 128], [1, 3]])
            nc.sync.dma_start(zpad_ap, zp[:])

        # ---- per-partition constants ----
        pidx = pers.tile([128, 1], I32)
        nc.gpsimd.iota(pidx[:], pattern=[[0, 1]], base=0, channel_multiplier=1)
        p16 = pers.tile([128, 1], I32)
        nc.vector.tensor_scalar(p16[:], pidx[:], 15, None, OP.bitwise_and)
        p32 = pers.tile([128, 1], I32)
        nc.vector.tensor_scalar(p32[:], pidx[:], 5, None, OP.arith_shift_right)
        pj16i = pers.tile([128, 1], I32)
        nc.vector.tensor_scalar(pj16i[:], p16[:], 3, None, OP.logical_shift_left)
        pj16f = pers.tile([128, 1], F32)
        nc.vector.tensor_copy(pj16f[:], pj16i[:])
        pg4i = pers.tile([128, 1], I32)
        nc.vector.tensor_scalar(pg4i[:], p32[:], 2, None, OP.logical_shift_left)
        pg4f = pers.tile([128, 1], F32)
        nc.vector.tensor_copy(pg4f[:], pg4i[:])
        p32m = pers.tile([128, 1], I32)
        nc.vector.tensor_scalar(p32m[:], pidx[:], 31, None, OP.bitwise_and)
        pp32i = pers.tile([128, 1], I32)
        nc.vector.tensor_scalar(pp32i[:], p32m[:], 3, None, OP.logical_shift_left)
        pp32f = pers.tile([128, 1], F32)
        nc.vector.tensor_copy(pp32f[:], pp32i[:])
        pg1f = pers.tile([128, 1], F32)
        nc.vector.tensor_copy(pg1f[:], p32[:])
        # wrapped i-base per chunk: pg4f + 128*chunk_half (precomputed)
        wib = []
        for ch in range(NCHPS):
            t = pers.tile([128, 1], F32, tag=f"wib{ch}")
            nc.vector.tensor_scalar(t[:], pg4f[:], float(128 * ch), None, OP.add)
            wib.append(t)

        # ---- hoisted iota bases (sample-independent) ----
        # wrapped: col = csg*64 + ci*8 + cqh*2 + cql
        # j = 128*cql + 8*(p%16) + ci ; i = 128*ch + 16*csg + 4*(p//32) + cqh
        wj0 = pers.tile([128, WCOLC], F32)
        nc.gpsimd.iota(wj0[:], pattern=[[0, CSG], [1, 8], [0, 4], [128, 2]],
                       base=0, channel_multiplier=0,
                       allow_small_or_imprecise_dtypes=True)
        wi0 = pers.tile([128, WCOLC], F32)
        nc.gpsimd.iota(wi0[:], pattern=[[16, CSG], [0, 8], [1, 4], [0, 2]],
                       base=0, channel_multiplier=0,
                       allow_small_or_imprecise_dtypes=True)
        # p-major (whole sample): col = ch*8 + k
        # j = 8*(p%32) + k ; i = 4*ch + p//32
        pj0 = pers.tile([128, FDWS], F32)
        nc.gpsimd.iota(pj0[:], pattern=[[0, FDWS // KPG], [1, KPG]],
                       base=0, channel_multiplier=0,
                       allow_small_or_imprecise_dtypes=True)
        pi0 = pers.tile([128, FDWS], F32)
        nc.gpsimd.iota(pi0[:], pattern=[[4, FDWS // KPG], [0, KPG]],
                       base=0, channel_multiplier=0,
                       allow_small_or_imprecise_dtypes=True)


        # ---- pools: phase-B pools allocated FIRST so the Z-build scratch
        # (zb) is address-disjoint — SBUF-reuse anti-deps would otherwise
        # serialize all of phase B behind the Z build ----
        wp = ctx.enter_context(tc.tile_pool(name="wp", bufs=1))
        ip = ctx.enter_context(tc.tile_pool(name="ip", bufs=2))     # w6d
        dp = ctx.enter_context(tc.tile_pool(name="dp", bufs=3))     # idxa
        gp = ctx.enter_context(tc.tile_pool(name="gp", bufs=4))     # ta halves
        fp = ctx.enter_context(tc.tile_pool(name="fp", bufs=4))     # f1
        op_ = ctx.enter_context(tc.tile_pool(name="op", bufs=4))    # ot
        zb = ctx.enter_context(tc.tile_pool(name="zb", bufs=2))



        # ================= Phase A: build Z (Sync DMA + ACT casts) =========
        def z_block(s, blk):
                rb = blk * RB
                last = blk == NBLK - 1
                nrows = RB if last else RB + 1
                tin = zb.tile([128, (RB + 1) * 128], F32, tag="tin")
                src = bass.AP(xp, s * SAMPLE_ELEMS + rb * W * C,
                              [[128, 128], [W * C, nrows], [1, 128]])
                tin_v = tin[:, 0:nrows * 128].rearrange(
                    "p (r q) -> p r q", r=nrows)
                nc.sync.dma_start(tin_v, src)
                if last:
                    # duplicate row 255 into slot RB (clip row)
                    dup = bass.AP(xp, s * SAMPLE_ELEMS + 255 * W * C,
                                  [[128, 128], [1, 128]])
                    nc.sync.dma_start(tin[:, RB * 128:(RB + 1) * 128], dup)
                zt = zb.tile([128, RB * 256], BF16, tag="zt")
                zt_t0 = bass.AP(zt[:].tensor, zt[:].offset,
                                [zt[:].ap[0], [256, RB], [128, 2], [1, 64]])
                zt_t1 = bass.AP(zt[:].tensor, zt[:].offset + 64,
                                [zt[:].ap[0], [256, RB], [128, 2], [1, 64]])
                src_r0 = bass.AP(tin[:].tensor, tin[:].offset,
                                 [tin[:].ap[0], [128, RB], [64, 2], [1, 64]])
                src_r1 = bass.AP(tin[:].tensor, tin[:].offset + 128,
                                 [tin[:].ap[0], [128, RB], [64, 2], [1, 64]])
                nc.scalar.copy(zt_t0, src_r0)
                nc.scalar.copy(zt_t1, src_r1)
                zdst = bass.AP(z, s * ZSTRIDE + rb * ZROW,
                               [[256, 128], [ZROW, RB], [1, 256]])
                nc.sync.dma_start(zdst, zt[:].rearrange(
                    "p (r q) -> p r q", r=RB))

        # ========== index/weight pipelines, all samples (DVE, upfront) =====
        # These run on the otherwise-idle DVE while the Z build streams
        # through Sync/ACT, so the Pool engine can start gathering the
        # moment sample 0's Z lands.

        def wrapped_pipe(s, ch):
            """Index pipeline for one chunk -> idxa tile. All-DVE (it gates
            the Pool gathers; engine crossings would add latency).

            Degenerate pixels (x-taps or y-taps clip to the same coord) have
            all-zero weights, so their gathered bytes are irrelevant — but
            clipped coords concentrate up to ~60% of a sample's descriptors
            onto ONE 768B entry, serializing a single HBM channel (measured
            2-3x slowdown on heavily-clipped samples). Redirect them to the
            identity-map entry (i_img*128 + own-xpair): uniform, sequential-
            friendly addresses."""
            wjf = wp.tile([128, WCOLC], F32, tag="jf")
            nc.vector.tensor_scalar(wjf[:], wj0[:], pj16f[:, 0:1], None,
                                    OP.add)
            wif = wp.tile([128, WCOLC], F32, tag="if")
            nc.vector.tensor_scalar(wif[:], wi0[:], wib[ch][:, 0:1], None,
                                    OP.add)
            wji = wp.tile([128, WCOLC], I32, tag="t_i")
            nc.vector.tensor_copy(wji[:], wjf[:])
            wjh = wp.tile([128, WCOLC], I32, tag="t_ax")
            nc.vector.tensor_scalar(wjh[:], wji[:], 1, None,
                                    OP.arith_shift_right)
            jhf = wp.tile([128, WCOLC], F32, tag="t_f")
            nc.vector.tensor_copy(jhf[:], wjh[:])
            jnk = wp.tile([128, WCOLC], F32, tag="wx1")
            nc.vector.scalar_tensor_tensor(jnk[:], wif[:], 128.0, jhf[:],
                                           OP.mult, OP.add)

            wpx, wpy = _coords(nc, wp, wjf, wif, th, s, "c", act=False)
            wx0f = _trunc(nc, wp, wpx, "t", act=False)
            wc0, wc1 = _clips(nc, wp, wx0f, "cx", act=False)
            _wxg, whgf = _hg(nc, wp, wc0, "hg", act=False)
            wy0f = _trunc(nc, wp, wpy, "t", act=False)
            wr0, wr1 = _clips(nc, wp, wy0f, "cy", act=False)

            gx = wp.tile([128, WCOLC], F32, tag="q")
            nc.vector.tensor_tensor(gx[:], wc1[:], wc0[:], OP.subtract)
            dy = wp.tile([128, WCOLC], F32, tag="g_")
            nc.vector.tensor_tensor(dy[:], wr1[:], wr0[:], OP.subtract)
            m = wp.tile([128, WCOLC], F32, tag="m")
            nc.vector.tensor_tensor(m[:], gx[:], dy[:], OP.mult)

            real = wp.tile([128, WCOLC], F32, tag="wx0a")
            nc.vector.scalar_tensor_tensor(real[:], wr0[:], 128.0, whgf[:],
                                           OP.mult, OP.add)
            dfj = wp.tile([128, WCOLC], F32, tag="wx1a")
            nc.vector.tensor_tensor(dfj[:], real[:], jnk[:], OP.subtract)
            mdf = wp.tile([128, WCOLC], F32, tag="wx0")
            nc.vector.tensor_tensor(mdf[:], m[:], dfj[:], OP.mult)
            idxa = dp.tile([128, WCOLC], I16, tag="idxa")
            nc.vector.tensor_tensor(idxa[:], mdf[:], jnk[:], OP.add)
            return idxa

        def pmajor_pipe(s):
            """Weight pipeline for a whole sample [128, FDWS] -> w6d."""
            pjf = wp.tile([128, FDWS], F32, tag="jf")
            nc.vector.tensor_scalar(pjf[:], pj0[:], pp32f[:, 0:1], None, OP.add)
            pif = wp.tile([128, FDWS], F32, tag="if")
            nc.vector.tensor_scalar(pif[:], pi0[:], pg1f[:, 0:1], None, OP.add)

            ppx, ppy = _coords(nc, wp, pjf, pif, th, s, "c", act=False)
            px0f = _trunc(nc, wp, ppx, "t", act=False)
            c0, c1 = _clips(nc, wp, px0f, "cx", act=False)
            xg, hgf = _hg(nc, wp, c0, "hg", act=False)
            py0f = _trunc(nc, wp, ppy, "t", act=False)
            r0, r1 = _clips(nc, wp, py0f, "cy", act=False)

            q = wp.tile([128, FDWS], F32, tag="q")
            nc.vector.scalar_tensor_tensor(q[:], hgf[:], -2.0, xg[:],
                                           OP.mult, OP.add)

            g_ = wp.tile([128, FDWS], F32, tag="g_")
            nc.vector.tensor_tensor(g_[:], c1[:], c0[:], OP.subtract)
            wx0a = wp.tile([128, FDWS], F32, tag="wx0a")
            nc.vector.tensor_tensor(wx0a[:], c1[:], ppx[:], OP.subtract)
            wx0 = wp.tile([128, FDWS], F32, tag="wx0")
            nc.vector.tensor_tensor(wx0[:], wx0a[:], g_[:], OP.mult)
            wx1a = wp.tile([128, FDWS], F32, tag="wx1a")
            nc.vector.tensor_tensor(wx1a[:], ppx[:], c0[:], OP.subtract)
            wx1 = wp.tile([128, FDWS], F32, tag="wx1")
            nc.vector.tensor_tensor(wx1[:], wx1a[:], g_[:], OP.mult)

            # q is exactly 0.0 or 1.0, so wx*(1-q) == wx - wx*q bit-for-bit.
            w012 = wp.tile([128, FDWS, 3], F32, tag="w012")
            nc.vector.tensor_tensor(w012[:, :, 1:2], wx0[:].unsqueeze(-1),
                                    q[:].unsqueeze(-1), OP.mult)
            nc.vector.tensor_tensor(w012[:, :, 2:3], wx1[:].unsqueeze(-1),
                                    q[:].unsqueeze(-1), OP.mult)
            nc.vector.tensor_tensor(w012[:, :, 0:1], wx0[:].unsqueeze(-1),
                                    w012[:, :, 1:2], OP.subtract)
            w1b = wp.tile([128, FDWS], F32, tag="w1b")
            nc.vector.tensor_tensor(w1b[:], wx1[:], w012[:, :, 2], OP.subtract)
            nc.vector.tensor_tensor(w012[:, :, 1:2], w012[:, :, 1:2],
                                    w1b[:].unsqueeze(-1), OP.add)

            # y weights with degenerate-row (r1==r0) redistribution:
            # d = r1-r0 in {0,1}; wy1' = wy1*d ; wy0' = wy0 + wy1*(1-d)
            wy0 = wp.tile([128, FDWS], F32, tag="wy0")
            nc.vector.tensor_tensor(wy0[:], r1[:], ppy[:], OP.subtract)
            wy1 = wp.tile([128, FDWS], F32, tag="wy1")
            nc.vector.tensor_tensor(wy1[:], ppy[:], r0[:], OP.subtract)
            d = wp.tile([128, FDWS], F32, tag="d")
            nc.vector.tensor_tensor(d[:], r1[:], r0[:], OP.subtract)
            e = wp.tile([128, FDWS], F32, tag="e")
            nc.vector.tensor_scalar(e[:], d[:], -1.0, 1.0, OP.mult, OP.add)
            m = wp.tile([128, FDWS], F32, tag="m")
            nc.vector.tensor_tensor(m[:], wy1[:], e[:], OP.mult)
            wy0p = wp.tile([128, FDWS], F32, tag="wy0p")
            nc.vector.tensor_tensor(wy0p[:], wy0[:], m[:], OP.add)
            wy1p = wp.tile([128, FDWS], F32, tag="wy1p")
            nc.vector.tensor_tensor(wy1p[:], wy1[:], d[:], OP.mult)

            # w6d[p, col, s*2+t, dup2] = w012[s] * wy't  (bf16, duplicated
            # pairwise so the value multiply's innermost dim is a unit-stride
            # 2-elem run on BOTH operands -> DVE 2x_1P mode)
            w6d = ip.tile([128, FDWS, 6, 2], BF16, tag="w6d")
            w6_t0 = bass.AP(w6d[:].tensor, w6d[:].offset,
                            [w6d[:].ap[0], [12, FDWS], [4, 3], [1, 2]])
            w6_t1 = bass.AP(w6d[:].tensor, w6d[:].offset + 2,
                            [w6d[:].ap[0], [12, FDWS], [4, 3], [1, 2]])
            w012_bc = bass.AP(w012[:].tensor, w012[:].offset,
                              [w012[:].ap[0], [3, FDWS], [1, 3], [0, 2]])
            wy0_bc = bass.AP(wy0p[:].tensor, wy0p[:].offset,
                             [wy0p[:].ap[0], [1, FDWS], [0, 3], [0, 2]])
            wy1_bc = bass.AP(wy1p[:].tensor, wy1p[:].offset,
                             [wy1p[:].ap[0], [1, FDWS], [0, 3], [0, 2]])
            nc.vector.tensor_tensor(w6_t0, w012_bc, wy0_bc, OP.mult)
            nc.vector.tensor_tensor(w6_t1, w012_bc, wy1_bc, OP.mult)
            return w6d

        # Issue order: Z(s0) blocks; s0 pipelines (DVE overlaps the Z-s0
        # DMA/casts); s0-ch0 compute with Z(s1) blocks interleaved into the
        # stream (so Z-s1's Sync/ACT queue entries don't head-of-line block
        # s0's output DMAs); s1 pipelines; remaining compute.
        for blk in range(NBLK):
            z_block(0, blk)

        idxas = {}
        idxas[(0, 0)] = wrapped_pipe(0, 0)
        w6d0 = pmajor_pipe(0)
        idxas[(0, 1)] = wrapped_pipe(0, 1)
        w6ds = {0: w6d0}

        zq = [(1, blk) for blk in range(NBLK)]  # remaining Z blocks

        def cs_group(s, ch, cs):
            cbase = s * NPIX_S + ch * CHUNK
            idxa = idxas[(s, ch)]
            w6d = w6ds[s]
            cs_s = ch * CSG + cs
            for hh in range(2):
                ta = gp.tile([128, 2 * KPG, 384], BF16, tag="ta")
                for gg in range(2):
                    g = hh * 2 + gg
                    nc.gpsimd.dma_gather(
                        out_ap=ta[:, gg * KPG:(gg + 1) * KPG, :],
                        in_ap=zaps[s],
                        idxs_ap=idxa[:, cs * 64:(cs + 1) * 64],
                        num_idxs=GN, num_idxs_reg=GN,
                        elem_size=384, elem_step=256, queue_num=g,
                    )
                # weight multiply at 2x: innermost 2-elem unit-stride run
                # on both operands (w6d is pair-duplicated)
                va5 = ta[:].rearrange("p k (sl c2 pr) -> p k sl c2 pr",
                                      sl=6, pr=2)
                lo = cs_s * 32 + hh * 16
                wsl = w6d[:, lo:lo + 16, :, :]
                w_bc = bass.AP(wsl.tensor, wsl.offset,
                               [wsl.ap[0], [12, 16], [2, 6], [0, 32], [1, 2]])
                nc.vector.tensor_tensor(va5, va5, w_bc, OP.mult)
                # reduce: t-pairs in place, then s-slots into f1 (frees ta
                # at the last DVE op — no ACT crossing holds the buffer)
                pdim = ta[:].ap[0]
                toff = ta[:].offset
                t0v = bass.AP(ta[:].tensor, toff,
                              [pdim, [384, 2 * KPG], [128, 3], [1, 64]])
                t1v = bass.AP(ta[:].tensor, toff + 64,
                              [pdim, [384, 2 * KPG], [128, 3], [1, 64]])
                nc.vector.tensor_tensor(t0v, t0v, t1v, OP.add)
                a0 = bass.AP(ta[:].tensor, toff,
                             [pdim, [384, 2 * KPG], [1, 64]])
                a1 = bass.AP(ta[:].tensor, toff + 128,
                             [pdim, [384, 2 * KPG], [1, 64]])
                a2 = bass.AP(ta[:].tensor, toff + 256,
                             [pdim, [384, 2 * KPG], [1, 64]])
                nc.vector.tensor_tensor(a0, a0, a1, OP.add)
                f1 = fp.tile([128, 2 * KPG, 64], BF16, tag="f1")
                nc.vector.tensor_tensor(f1[:], a0, a2, OP.add)
                ot = op_.tile([128, 2 * KPG, 64], F32, tag="ot")
                nc.scalar.copy(ot[:], f1[:])

                # out rows: cbase + cs*GROUP + (hh*2+gg)*GN + p*KPG + k
                obase = cbase + cs * GROUP + hh * 2 * GN
                oap = bass.AP(out_d.tensor, obase * C,
                              [[KPG * C, 128], [GN * C, 2], [1, KPG * C]])
                ovw = ot[:].rearrange("p (g k) c -> p g (k c)", g=2)
                nc.scalar.dma_start(oap, ovw)

        zper = (len(zq) + CSG - 1) // CSG
        for cs in range(CSG):
            cs_group(0, 0, cs)
            for _ in range(zper):
                if zq:
                    z_block(*zq.pop(0))

        idxas[(1, 0)] = wrapped_pipe(1, 0)
        w6ds[1] = pmajor_pipe(1)
        idxas[(1, 1)] = wrapped_pipe(1, 1)

        for cs in range(CSG):
            cs_group(0, 1, cs)
        for ch in range(NCHPS):
            for cs in range(CSG):
                cs_group(1, ch, cs)

    nc.compile()
    return nc


def _get_runner():
    """Build once: the Bass program, the sharded jitted executor, and the
    device-resident zero output buffers. Cached for repeat kernel() calls."""
    if "runner" in _cached:
        return _cached["runner"]

    import jax
    from jax.sharding import Mesh, PartitionSpec, NamedSharding
    from jax.experimental.shard_map import shard_map
    import concourse.bass2jax as bass2jax

    nc = build()
    bass2jax.install_neuronx_cc_hook()

    in_names, out_names, out_avals, zero_outs = [], [], [], []
    pn = nc.partition_id_tensor.name if nc.partition_id_tensor else None
    for alloc in nc.m.functions[0].allocations:
        if not isinstance(alloc, mybir.MemoryLocationSet):
            continue
        name = alloc.memorylocations[0].name
        if alloc.kind == "ExternalInput":
            if name != pn:
                in_names.append(name)
        elif alloc.kind == "ExternalOutput":
            out_names.append(name)
            shape = tuple(alloc.tensor_shape)
            dtype = mybir.dt.np(alloc.dtype)
            out_avals.append(jax.core.ShapedArray(shape, dtype))
            zero_outs.append(np.zeros(shape, dtype))

    def _body(*args):
        ops = list(args)
        if pn is not None:
            ops.append(bass2jax.partition_id_tensor())
        return tuple(bass2jax._bass_exec_p.bind(
            *ops,
            out_avals=tuple(out_avals),
            in_names=tuple(list(in_names) + out_names + ([pn] if pn else [])),
            out_names=tuple(out_names),
            lowering_input_output_aliases=(),
            sim_require_finite=True,
            sim_require_nnan=True,
            nc=nc,
        ))

    devices = jax.devices()[:N_CORES]
    mesh = Mesh(np.asarray(devices), ("core",))
    nin = len(in_names) + len(out_names)
    fn = jax.jit(
        shard_map(_body, mesh=mesh, in_specs=(PartitionSpec("core"),) * nin,
                  out_specs=(PartitionSpec("core"),) * len(out_names),
                  check_rep=False),
        keep_unused=True,
    )
    sh = NamedSharding(mesh, PartitionSpec("core"))
    dz = [jax.device_put(np.zeros((N_CORES * z.shape[0], *z.shape[1:]), z.dtype), sh)
          for z in zero_outs]
    runner = {
        "fn": fn, "dz": dz, "sh": sh, "in_names": in_names,
        "out_idx": out_names.index("out"), "device_put": jax.device_put,
        "nc": nc,
    }
    _cached["runner"] = runner
    return runner


def kernel(X, theta):
    X = np.ascontiguousarray(X, dtype=np.float32)
    theta = np.ascontiguousarray(theta, dtype=np.float32)
    r = _get_runner()
    per_name = {
        "xp": X.reshape(N_CORES, B_PER_CORE * SAMPLE_ELEMS).reshape(-1),
        "th": theta.reshape(N_CORES * B_PER_CORE, 6),
    }
    di = [r["device_put"](per_name[nm], r["sh"]) for nm in r["in_names"]]
    out = r["fn"](*di, *r["dz"])
    res = np.asarray(out[r["out_idx"]])
    return res.reshape(N_CORES * B_PER_CORE, H, W, C)

